# revision 12
# baseline (speedup 1.0000x reference)
"""Tensor-parallel GQA attention layer for 8 Trainium2 NeuronCores.

Shapes (hardcoded from the problem spec):
  x [1, 2048, 4096] f32, wq [4096, 4096], wk/wv [1024, 4096],
  wo [4096, 4096], freqs_cos/sin [2048, 64], mask [2048, 2048].

Sharding: tensor-parallel over heads. Core i owns q-heads 4i..4i+3 and
kv-head i (wq/wk/wv column-parallel). The output projection is sharded
over OUTPUT columns instead of rows: head outputs are AllGathered
(2MB/core) and each core computes out[:, 512i:512(i+1)], avoiding the
32MB all-reduce a row-parallel wo would need.

Numerics: matmuls in bf16 (fp32 PSUM accumulation); softmax in fp32 on
the scalar engine without max-subtraction (scores are O(1) by
construction); masking via elementwise multiply with exp(mask), applied
only to tiles where exp(mask) is neither all-ones nor all-zero
(all-zero tiles are skipped entirely, which for the causal mask removes
~38% of attention work).
"""

import math
import sys

for _p in ("/opt/trn_rl_repo",):
    if _p not in sys.path:
        sys.path.append(_p)

import numpy as np
import ml_dtypes

import concourse.bass as bass
import concourse.mybir as mybir
import concourse.tile as tile
from concourse.bass_utils import run_bass_kernel_spmd
from concourse.masks import make_identity
from concourse.vector_clock import ScopedClock

BF16 = mybir.dt.bfloat16
F32 = mybir.dt.float32
AF = mybir.ActivationFunctionType

N_CORES = 8
DIM = 4096
SEQ = 2048
HD = 128                      # head dim == partition dim
NQH = 4                       # q heads per core
P = 128
SC = 512                      # seq chunk (psum bank free size in f32)
ND = DIM // P                 # 32 contraction tiles
NSC = SEQ // SC               # 4 seq chunks
NKT = SEQ // P                # 16 k tiles
QCOLS = NQH * HD              # 512 q columns per core

LAST_RESULT = None            # BassKernelResults of the most recent kernel() call


def _patch_tile_drain():
    """The walrus build in this container rejects Drain instructions that
    carry more than one sync-wait (and sem-eq waits). Spread the tile-exit
    waits across single-wait nops and use sem-only barriers instead."""

    def patched(self, tick_clock, wait_clock):
        carrier = self.nc.sync.nop(nofuse=True)
        wait_clock.add_sem_waits(
            carrier.ins, ScopedClock({None: tick_clock.global_clock})
        )
        si = carrier.ins.sync_info
        waits = list(si.on_wait) if si and si.on_wait else []
        if len(waits) > 1:
            si.on_wait = waits[:1]
            for w in waits[1:]:
                extra = self.nc.sync.nop(nofuse=True)
                extra.ins.sync_info = mybir.SyncInfo(on_wait=[w], on_update=[])
        self.nc.sync.drain()
        self.nc.all_engine_barrier(sem_only=True)
        popped = self.nc._tile_sem_poison_stack.pop()
        assert popped is self._sem_poison
        self.nc.clear_and_free_semaphores(list(self.sems.allocated().values()))
        self.nc.all_engine_barrier(sem_only=True)

    tile.TileContext._drain_and_barrier = patched


_patch_tile_drain()


def _split_multi_waits(nc, limit=1):
    """This walrus build supports ~one sync-wait per instruction (and none
    on Drain). Hoist excess waits onto single-wait NoOps inserted just
    before the instruction on the same engine queue (FIFO => equivalent)."""
    for fn in nc.m.functions:
        for bb in fn.blocks:
            out = []
            changed = False
            for ins in bb.instructions:
                si = getattr(ins, "sync_info", None)
                waits = list(si.on_wait) if si is not None and si.on_wait else []
                keep = 0 if type(ins).__name__ == "InstDrain" else limit
                if len(waits) > keep:
                    changed = True
                    for w in waits[keep:]:
                        nop = mybir.InstNoOp(
                            name=f"WSPLIT-{nc.next_id()}", ins=[], outs=[])
                        nop.engine = ins.engine
                        nop.sync_info = mybir.SyncInfo(on_wait=[w], on_update=[])
                        out.append(nop)
                    si.on_wait = waits[:keep]
                out.append(ins)
            if changed:
                bb.instructions[:] = out


def _classify_mask(mask):
    """Per (ki, qj) tile classes of exp(mask).T: 0=no-op, 1=multiply, 2=skip.

    Returns (classes [NKT, NSC], packed mixed tiles [n_mixed*P, SC] bf16,
    mixed index map {(ki, qj): packed_idx}).
    """
    em = np.exp(mask.astype(np.float64)).astype(np.float32).T  # [k, q]
    classes = np.zeros((NKT, NSC), dtype=np.int32)
    mixed = []
    mixed_idx = {}
    for ki in range(NKT):
        for qj in range(NSC):
            t = em[ki * P:(ki + 1) * P, qj * SC:(qj + 1) * SC]
            if np.all(t == 1.0):
                classes[ki, qj] = 0
            elif np.all(t == 0.0):
                classes[ki, qj] = 2
            else:
                classes[ki, qj] = 1
                mixed_idx[(ki, qj)] = len(mixed)
                mixed.append(t.astype(ml_dtypes.bfloat16))
    if mixed:
        packed = np.concatenate(mixed, axis=0)
    else:
        packed = np.zeros((P, SC), dtype=ml_dtypes.bfloat16)
    return classes, packed, mixed_idx


def _build_program(classes, mixed_idx, n_mixed):
    nc = bass.Bass()

    xT_d = nc.dram_tensor("xT", [DIM, SEQ], BF16, kind="ExternalInput")
    wqT_d = nc.dram_tensor("wqT", [DIM, QCOLS], BF16, kind="ExternalInput")
    wkT_d = nc.dram_tensor("wkT", [DIM, HD], BF16, kind="ExternalInput")
    wvT_d = nc.dram_tensor("wvT", [DIM, HD], BF16, kind="ExternalInput")
    woT_d = nc.dram_tensor("woT", [DIM, QCOLS], BF16, kind="ExternalInput")
    ropeC_d = nc.dram_tensor("ropeC", [P, SEQ], F32, kind="ExternalInput")
    ropeS_d = nc.dram_tensor("ropeS", [P, SEQ], F32, kind="ExternalInput")
    mm_d = nc.dram_tensor("maskmul", [max(n_mixed, 1) * P, SC], BF16,
                          kind="ExternalInput")
    out_d = nc.dram_tensor("out", [SEQ, QCOLS], F32, kind="ExternalOutput")

    scale = 1.0 / math.sqrt(HD)
    MUL = mybir.AluOpType.mult

    with tile.TileContext(nc) as tc:
        with tc.tile_pool(name="const", bufs=1) as cp, \
             tc.tile_pool(name="acts", bufs=1) as ap, \
             tc.tile_pool(name="wo", bufs=1) as wop, \
             tc.tile_pool(name="ahp", bufs=1) as ahp:
            ident = cp.tile([P, P], BF16, tag="ident", name="ident")
            make_identity(nc, ident[:])
            ones_col = cp.tile([P, 1], BF16, tag="ones_col", name="ones_col")
            nc.gpsimd.memset(ones_col[:], 1.0)
            ones_row = cp.tile([1, P], F32, tag="ones_row", name="ones_row")
            nc.gpsimd.memset(ones_row[:], 1.0)
            qT = [ap.tile([P, SEQ], BF16, tag=f"qT{h}", name=f"qT{h}") for h in range(NQH)]
            kT = ap.tile([P, SEQ], BF16, tag="kT", name="kT")
            V = [ap.tile([P, HD], BF16, tag=f"V{t}", name=f"V{t}") for t in range(NKT)]
            attnT = [ap.tile([P, SEQ], BF16, tag=f"attnT{h}", name=f"attnT{h}") for h in range(NQH)]

            wo_sb = [wop.tile([P, QCOLS], BF16, tag=f"wo{c}", name=f"wo{c}") for c in range(ND)]
            for c in range(ND):
                nc.sync.dma_start(wo_sb[c][:], woT_d[c * P:(c + 1) * P, :])

            # ---- phase 1: QKV projections + RoPE + V transpose ----
            with tc.tile_pool(name="w1", bufs=1) as wp, \
                 tc.tile_pool(name="xs", bufs=3) as xp, \
                 tc.tile_pool(name="rtmp", bufs=2) as rp, \
                 tc.tile_pool(name="pq", bufs=1, space="PSUM") as pqp, \
                 tc.tile_pool(name="pkv", bufs=1, space="PSUM") as pkvp, \
                 tc.tile_pool(name="ptr", bufs=2, space="PSUM") as ptrp:
                ropeC = wp.tile([P, SEQ], F32, tag="ropeC", name="ropeC")
                nc.sync.dma_start(ropeC[:], ropeC_d[:])
                ropeS = wp.tile([P, SEQ], F32, tag="ropeS", name="ropeS")
                nc.sync.dma_start(ropeS[:], ropeS_d[:])
                wq_sb = [wp.tile([P, QCOLS], BF16, tag=f"wq{d}", name=f"wq{d}") for d in range(ND)]
                wk_sb = [wp.tile([P, HD], BF16, tag=f"wk{d}", name=f"wk{d}") for d in range(ND)]
                wv_sb = [wp.tile([P, HD], BF16, tag=f"wv{d}", name=f"wv{d}") for d in range(ND)]
                for d in range(ND):
                    nc.sync.dma_start(wq_sb[d][:], wqT_d[d * P:(d + 1) * P, :])
                    nc.sync.dma_start(wk_sb[d][:], wkT_d[d * P:(d + 1) * P, :])
                    nc.sync.dma_start(wv_sb[d][:], wvT_d[d * P:(d + 1) * P, :])

                for sc in range(NSC):
                    ssl = slice(sc * SC, (sc + 1) * SC)
                    psq = [pqp.tile([P, SC], F32, tag=f"psq{h}", name=f"psq{h}") for h in range(NQH)]
                    psk = pkvp.tile([P, SC], F32, tag="psk", name="psk")
                    psv = pkvp.tile([P, SC], F32, tag="psv", name="psv")
                    for d in range(ND):
                        xt = xp.tile([P, SC], BF16, tag="xt", name="xt")
                        nc.sync.dma_start(xt[:], xT_d[d * P:(d + 1) * P, ssl])
                        st, sp = d == 0, d == ND - 1
                        for h in range(NQH):
                            nc.tensor.matmul(
                                psq[h][:], wq_sb[d][:, h * HD:(h + 1) * HD],
                                xt[:], start=st, stop=sp)
                        nc.tensor.matmul(psk[:], wk_sb[d][:], xt[:], start=st, stop=sp)
                        nc.tensor.matmul(psv[:], wv_sb[d][:], xt[:], start=st, stop=sp)

                    # RoPE on q heads and k (deinterleaved pair layout:
                    # swap partition halves, multiply by host-built tables)
                    H = P // 2
                    for h in range(NQH + 1):
                        src = psk if h == NQH else psq[h]
                        dst = kT if h == NQH else qT[h]
                        tsw = rp.tile([P, SC], F32, tag="tsw", name="tsw")
                        nc.scalar.activation(tsw[0:H, :], src[H:P, :], AF.Copy)
                        nc.scalar.activation(tsw[H:P, :], src[0:H, :], AF.Copy)
                        t1 = rp.tile([P, SC], F32, tag="t1", name="t1")
                        nc.vector.tensor_mul(t1[:], src[:], ropeC[:, ssl])
                        t2 = rp.tile([P, SC], F32, tag="t2", name="t2")
                        nc.vector.tensor_mul(t2[:], tsw[:], ropeS[:, ssl])
                        nc.vector.tensor_add(dst[:, ssl], t1[:], t2[:])

                    # V: transpose [hd, s] chunk into [s, hd] tiles
                    vtmp = rp.tile([P, SC], BF16, tag="vtmp", name="vtmp")
                    nc.scalar.activation(vtmp[:], psv[:], AF.Copy)
                    for t in range(SC // P):
                        ptr = ptrp.tile([P, P], BF16, tag="ptr", name="ptr")
                        nc.tensor.transpose(
                            ptr[:], vtmp[:, t * P:(t + 1) * P], ident[:])
                        nc.scalar.activation(
                            V[sc * (SC // P) + t][:], ptr[:], AF.Copy)

            # ---- phase 2: attention (transposed scores), AllGather per head ----
            dp = tc.alloc_tile_pool(name="dram", bufs=1, space="DRAM")
            with tc.tile_pool(name="mmul", bufs=1) as mmp, \
                 tc.tile_pool(name="E", bufs=2) as ep, \
                 tc.tile_pool(name="r2", bufs=2) as r2p, \
                 tc.tile_pool(name="pss", bufs=2, space="PSUM") as pssp, \
                 tc.tile_pool(name="psum", bufs=2, space="PSUM") as psump, \
                 tc.tile_pool(name="pav", bufs=2, space="PSUM") as pavp, \
                 tc.tile_pool(name="pb", bufs=1, space="PSUM") as pbp:
                cc_in = [dp.tile([P, SEQ], BF16, tag=f"cc_in{h}", name=f"cc_in{h}")
                         for h in range(NQH)]
                cc_out = [dp.tile([N_CORES * P, SEQ], BF16,
                                  tag=f"cc_out{h}", name=f"cc_out{h}")
                          for h in range(NQH)]
                mask_sb = [mmp.tile([P, SC], BF16, tag=f"mm{i}", name=f"mm{i}")
                           for i in range(max(n_mixed, 1))]
                for i in range(n_mixed):
                    nc.sync.dma_start(mask_sb[i][:], mm_d[i * P:(i + 1) * P, :])

                for h in range(NQH):
                    for qj in range(NSC):
                        qsl = slice(qj * SC, (qj + 1) * SC)
                        live = [ki for ki in range(NKT) if classes[ki, qj] != 2]
                        Es = []
                        for ki in live:
                            pss = pssp.tile([P, SC], F32, tag="pss", name="pss")
                            nc.tensor.matmul(
                                pss[:], kT[:, ki * P:(ki + 1) * P],
                                qT[h][:, qsl], start=True, stop=True)
                            e = ep.tile([P, SC], BF16, tag=f"E{ki}", name=f"E{ki}")
                            nc.scalar.activation(e[:], pss[:], AF.Exp, scale=scale)
                            if classes[ki, qj] == 1:
                                nc.vector.tensor_mul(
                                    e[:], e[:], mask_sb[mixed_idx[(ki, qj)]][:])
                            Es.append((ki, e))
                        psum = psump.tile([1, SC], F32, tag="psum", name="psum")
                        for i, (ki, e) in enumerate(Es):
                            nc.tensor.matmul(psum[:], ones_col[:], e[:],
                                             start=i == 0, stop=i == len(Es) - 1)
                        pav = pavp.tile([P, SC], F32, tag="pav", name="pav")
                        for i, (ki, e) in enumerate(Es):
                            nc.tensor.matmul(pav[:], V[ki][:], e[:],
                                             start=i == 0, stop=i == len(Es) - 1)
                        sums = r2p.tile([1, SC], F32, tag="sums", name="sums")
                        nc.scalar.activation(sums[:], psum[:], AF.Copy)
                        pb = pbp.tile([P, SC], F32, tag="pb", name="pb")
                        nc.tensor.matmul(pb[:], ones_row[:], sums[:],
                                         start=True, stop=True)
                        bsb = r2p.tile([P, SC], F32, tag="bsb", name="bsb")
                        nc.vector.reciprocal(bsb[:], pb[:])
                        nc.vector.tensor_mul(attnT[h][:, qsl], pav[:], bsb[:])
                    # ship this head while later heads compute (gpsimd/SWDGE:
                    # HWDGE DMAs concurrent with collectives hang in NRT)
                    nc.gpsimd.dma_start(cc_in[h][:], attnT[h][:])
                    nc.gpsimd.collective_compute(
                        "AllGather", mybir.AluOpType.bypass,
                        replica_groups=[list(range(N_CORES))],
                        ins=[cc_in[h].opt()], outs=[cc_out[h].opt()])

            # ---- phase 3: output projection columns ----
            # cc_out[h] row-tile j holds global head 4*j + h
            with tc.tile_pool(name="po", bufs=2, space="PSUM") as pop, \
                 tc.tile_pool(name="ob", bufs=2) as obp:
                NW = SEQ // SC
                for w in range(4):
                    wsl = slice(w * SC, (w + 1) * SC)
                    ah = [ahp.tile([P, SC], BF16, tag=f"ah{c}", name=f"ah{c}")
                          for c in range(ND)]
                    for h in range(NQH):
                        for j in range(N_CORES):
                            nc.sync.dma_start(
                                ah[NQH * j + h][:],
                                cc_out[h][j * P:(j + 1) * P, wsl])
                    for s4 in range(SC // P):
                        st = w * (SC // P) + s4
                        po = pop.tile([P, QCOLS], F32, tag="po", name="po")
                        for c in range(ND):
                            nc.tensor.matmul(
                                po[:], ah[c][:, s4 * P:(s4 + 1) * P],
                                wo_sb[c][:], start=c == 0, stop=c == ND - 1)
                        ob = obp.tile([P, QCOLS], F32, tag="ob", name="ob")
                        nc.scalar.activation(ob[:], po[:], AF.Copy)
                        nc.sync.dma_start(out_d[st * P:(st + 1) * P, :], ob[:])
            dp.release()

    _split_multi_waits(nc)
    return nc


def kernel(x, wq, wk, wv, wo, freqs_cos, freqs_sin, mask):
    x = np.asarray(x, dtype=np.float32)
    wq = np.asarray(wq, dtype=np.float32)
    wk = np.asarray(wk, dtype=np.float32)
    wv = np.asarray(wv, dtype=np.float32)
    wo = np.asarray(wo, dtype=np.float32)
    freqs_cos = np.asarray(freqs_cos, dtype=np.float32)
    freqs_sin = np.asarray(freqs_sin, dtype=np.float32)
    mask = np.asarray(mask, dtype=np.float32)

    bf = ml_dtypes.bfloat16
    # deinterleave head_dim pairs so RoPE becomes a partition-half swap
    perm = np.concatenate([np.arange(0, HD, 2), np.arange(1, HD, 2)])
    wq_p = wq.reshape(-1, HD, DIM)[:, perm, :].reshape(wq.shape)
    wk_p = wk.reshape(-1, HD, DIM)[:, perm, :].reshape(wk.shape)

    xT = np.ascontiguousarray(x[0].T).astype(bf)               # [DIM, SEQ]
    ropeC = np.ascontiguousarray(
        np.concatenate([freqs_cos.T, freqs_cos.T], axis=0))     # [128, SEQ]
    ropeS = np.ascontiguousarray(
        np.concatenate([-freqs_sin.T, freqs_sin.T], axis=0))

    classes, maskpack, mixed_idx = _classify_mask(mask)
    n_mixed = len(mixed_idx)

    nc = _build_program(classes, mixed_idx, n_mixed)

    in_maps = []
    for i in range(N_CORES):
        wqT = np.ascontiguousarray(
            wq_p[i * QCOLS:(i + 1) * QCOLS, :].T).astype(bf)    # [DIM, 512]
        wkT = np.ascontiguousarray(
            wk_p[i * HD:(i + 1) * HD, :].T).astype(bf)          # [DIM, 128]
        wvT = np.ascontiguousarray(
            wv[i * HD:(i + 1) * HD, :].T).astype(bf)
        # out[:, 512i:512(i+1)] = attn_full @ wo.T[:, 512i:...]
        woT = np.ascontiguousarray(
            wo[i * QCOLS:(i + 1) * QCOLS, :].T).astype(bf)      # [DIM, 512]
        in_maps.append({
            "xT": xT, "wqT": wqT, "wkT": wkT, "wvT": wvT, "woT": woT,
            "ropeC": ropeC, "ropeS": ropeS, "maskmul": maskpack,
        })

    res = run_bass_kernel_spmd(nc, in_maps, list(range(N_CORES)))
    global LAST_RESULT
    LAST_RESULT = res
    out = np.concatenate(
        [np.asarray(res.results[i]["out"]) for i in range(N_CORES)], axis=1)
    return out.reshape(1, SEQ, DIM).astype(np.float32)


# revision 13
# speedup vs baseline: 1.0796x; 1.0796x over previous
"""Tensor-parallel GQA attention layer for 8 Trainium2 NeuronCores.

Shapes (hardcoded from the problem spec):
  x [1, 2048, 4096] f32, wq [4096, 4096], wk/wv [1024, 4096],
  wo [4096, 4096], freqs_cos/sin [2048, 64], mask [2048, 2048].

Sharding: tensor-parallel over heads. Core i owns q-heads 4i..4i+3 and
kv-head i (wq/wk/wv column-parallel). The output projection is sharded
over OUTPUT columns instead of rows: head outputs are AllGathered
(2MB/core) and each core computes out[:, 512i:512(i+1)], avoiding the
32MB all-reduce a row-parallel wo would need.

Numerics: matmuls in bf16 (fp32 PSUM accumulation); softmax in fp32 on
the scalar engine without max-subtraction (scores are O(1) by
construction); masking via elementwise multiply with exp(mask), applied
only to tiles where exp(mask) is neither all-ones nor all-zero
(all-zero tiles are skipped entirely, which for the causal mask removes
~38% of attention work).
"""

import math
import sys

for _p in ("/opt/trn_rl_repo",):
    if _p not in sys.path:
        sys.path.append(_p)

import numpy as np
import ml_dtypes

import concourse.bass as bass
import concourse.mybir as mybir
import concourse.tile as tile
from concourse.bass_utils import run_bass_kernel_spmd
from concourse.masks import make_identity
from concourse.vector_clock import ScopedClock

BF16 = mybir.dt.bfloat16
F32 = mybir.dt.float32
AF = mybir.ActivationFunctionType

N_CORES = 8
DIM = 4096
SEQ = 2048
HD = 128                      # head dim == partition dim
NQH = 4                       # q heads per core
P = 128
SC = 512                      # seq chunk (psum bank free size in f32)
ND = DIM // P                 # 32 contraction tiles
NSC = SEQ // SC               # 4 seq chunks
NKT = SEQ // P                # 16 k tiles
QCOLS = NQH * HD              # 512 q columns per core

LAST_RESULT = None            # BassKernelResults of the most recent kernel() call


def _patch_tile_drain():
    """The walrus build in this container rejects Drain instructions that
    carry more than one sync-wait (and sem-eq waits). Spread the tile-exit
    waits across single-wait nops and use sem-only barriers instead."""

    def patched(self, tick_clock, wait_clock):
        carrier = self.nc.sync.nop(nofuse=True)
        wait_clock.add_sem_waits(
            carrier.ins, ScopedClock({None: tick_clock.global_clock})
        )
        si = carrier.ins.sync_info
        waits = list(si.on_wait) if si and si.on_wait else []
        if len(waits) > 1:
            si.on_wait = waits[:1]
            for w in waits[1:]:
                extra = self.nc.sync.nop(nofuse=True)
                extra.ins.sync_info = mybir.SyncInfo(on_wait=[w], on_update=[])
        self.nc.sync.drain()
        self.nc.all_engine_barrier(sem_only=True)
        popped = self.nc._tile_sem_poison_stack.pop()
        assert popped is self._sem_poison
        self.nc.clear_and_free_semaphores(list(self.sems.allocated().values()))
        self.nc.all_engine_barrier(sem_only=True)

    tile.TileContext._drain_and_barrier = patched


_patch_tile_drain()


def _split_multi_waits(nc, limit=1):
    """This walrus build supports ~one sync-wait per instruction (and none
    on Drain). Hoist excess waits onto single-wait NoOps inserted just
    before the instruction on the same engine queue (FIFO => equivalent)."""
    for fn in nc.m.functions:
        for bb in fn.blocks:
            out = []
            changed = False
            for ins in bb.instructions:
                si = getattr(ins, "sync_info", None)
                waits = list(si.on_wait) if si is not None and si.on_wait else []
                keep = 0 if type(ins).__name__ == "InstDrain" else limit
                if len(waits) > keep:
                    changed = True
                    for w in waits[keep:]:
                        nop = mybir.InstNoOp(
                            name=f"WSPLIT-{nc.next_id()}", ins=[], outs=[])
                        nop.engine = ins.engine
                        nop.sync_info = mybir.SyncInfo(on_wait=[w], on_update=[])
                        out.append(nop)
                    si.on_wait = waits[:keep]
                out.append(ins)
            if changed:
                bb.instructions[:] = out


def _classify_mask(mask):
    """Per (ki, qj) tile classes of exp(mask).T: 0=no-op, 1=multiply, 2=skip.

    Returns (classes [NKT, NSC], packed mixed tiles [n_mixed*P, SC] bf16,
    mixed index map {(ki, qj): packed_idx}).
    """
    em = np.exp(mask.astype(np.float64)).astype(np.float32).T  # [k, q]
    classes = np.zeros((NKT, NSC), dtype=np.int32)
    mixed = []
    mixed_idx = {}
    for ki in range(NKT):
        for qj in range(NSC):
            t = em[ki * P:(ki + 1) * P, qj * SC:(qj + 1) * SC]
            if np.all(t == 1.0):
                classes[ki, qj] = 0
            elif np.all(t == 0.0):
                classes[ki, qj] = 2
            else:
                classes[ki, qj] = 1
                mixed_idx[(ki, qj)] = len(mixed)
                mixed.append(t.astype(ml_dtypes.bfloat16))
    if mixed:
        packed = np.concatenate(mixed, axis=0)
    else:
        packed = np.zeros((P, SC), dtype=ml_dtypes.bfloat16)
    return classes, packed, mixed_idx


def _build_program(classes, mixed_idx, n_mixed):
    nc = bass.Bass()

    xT_d = nc.dram_tensor("xT", [DIM, SEQ], BF16, kind="ExternalInput")
    wqT_d = nc.dram_tensor("wqT", [DIM, QCOLS], BF16, kind="ExternalInput")
    wkT_d = nc.dram_tensor("wkT", [DIM, HD], BF16, kind="ExternalInput")
    wvT_d = nc.dram_tensor("wvT", [DIM, HD], BF16, kind="ExternalInput")
    woT_d = nc.dram_tensor("woT", [DIM, QCOLS], BF16, kind="ExternalInput")
    ropeC_d = nc.dram_tensor("ropeC", [P, SEQ], F32, kind="ExternalInput")
    ropeS_d = nc.dram_tensor("ropeS", [P, SEQ], F32, kind="ExternalInput")
    mm_d = nc.dram_tensor("maskmul", [max(n_mixed, 1) * P, SC], BF16,
                          kind="ExternalInput")
    out_d = nc.dram_tensor("out", [SEQ, QCOLS], F32, kind="ExternalOutput")

    scale = 1.0 / math.sqrt(HD)
    MUL = mybir.AluOpType.mult

    with tile.TileContext(nc) as tc:
        with tc.tile_pool(name="const", bufs=1) as cp, \
             tc.tile_pool(name="acts", bufs=1) as ap, \
             tc.tile_pool(name="wo", bufs=1) as wop, \
             tc.tile_pool(name="ahp", bufs=1) as ahp:
            ident = cp.tile([P, P], BF16, tag="ident", name="ident")
            make_identity(nc, ident[:])
            ones_col = cp.tile([P, 1], BF16, tag="ones_col", name="ones_col")
            nc.gpsimd.memset(ones_col[:], 1.0)
            ones_row = cp.tile([1, P], F32, tag="ones_row", name="ones_row")
            nc.gpsimd.memset(ones_row[:], 1.0)
            qT = [ap.tile([P, SEQ], BF16, tag=f"qT{h}", name=f"qT{h}") for h in range(NQH)]
            kT = ap.tile([P, SEQ], BF16, tag="kT", name="kT")
            V = [ap.tile([P, HD], BF16, tag=f"V{t}", name=f"V{t}") for t in range(NKT)]
            attnT = [ap.tile([P, SEQ], BF16, tag=f"attnT{h}", name=f"attnT{h}") for h in range(NQH)]

            wo_sb = [wop.tile([P, QCOLS], BF16, tag=f"wo{c}", name=f"wo{c}") for c in range(ND)]
            for c in range(ND):
                nc.sync.dma_start(wo_sb[c][:], woT_d[c * P:(c + 1) * P, :])

            # ---- phase 1: QKV projections + RoPE + V transpose ----
            with tc.tile_pool(name="w1", bufs=1) as wp, \
                 tc.tile_pool(name="xs", bufs=3) as xp, \
                 tc.tile_pool(name="rtmp", bufs=2) as rp, \
                 tc.tile_pool(name="pq", bufs=1, space="PSUM") as pqp, \
                 tc.tile_pool(name="pkv", bufs=1, space="PSUM") as pkvp, \
                 tc.tile_pool(name="ptr", bufs=2, space="PSUM") as ptrp:
                ropeC = wp.tile([P, SEQ], F32, tag="ropeC", name="ropeC")
                nc.sync.dma_start(ropeC[:], ropeC_d[:])
                ropeS = wp.tile([P, SEQ], F32, tag="ropeS", name="ropeS")
                nc.sync.dma_start(ropeS[:], ropeS_d[:])
                wq_sb = [wp.tile([P, QCOLS], BF16, tag=f"wq{d}", name=f"wq{d}") for d in range(ND)]
                wk_sb = [wp.tile([P, HD], BF16, tag=f"wk{d}", name=f"wk{d}") for d in range(ND)]
                wv_sb = [wp.tile([P, HD], BF16, tag=f"wv{d}", name=f"wv{d}") for d in range(ND)]
                for d in range(ND):
                    nc.sync.dma_start(wq_sb[d][:], wqT_d[d * P:(d + 1) * P, :])
                    nc.sync.dma_start(wk_sb[d][:], wkT_d[d * P:(d + 1) * P, :])
                    nc.sync.dma_start(wv_sb[d][:], wvT_d[d * P:(d + 1) * P, :])

                for sc in range(NSC):
                    ssl = slice(sc * SC, (sc + 1) * SC)
                    psq = [pqp.tile([P, SC], F32, tag=f"psq{h}", name=f"psq{h}") for h in range(NQH)]
                    psk = pkvp.tile([P, SC], F32, tag="psk", name="psk")
                    psv = pkvp.tile([P, SC], F32, tag="psv", name="psv")
                    for d in range(ND):
                        xt = xp.tile([P, SC], BF16, tag="xt", name="xt")
                        nc.sync.dma_start(xt[:], xT_d[d * P:(d + 1) * P, ssl])
                        st, sp = d == 0, d == ND - 1
                        for h in range(NQH):
                            nc.tensor.matmul(
                                psq[h][:], wq_sb[d][:, h * HD:(h + 1) * HD],
                                xt[:], start=st, stop=sp)
                        nc.tensor.matmul(psk[:], wk_sb[d][:], xt[:], start=st, stop=sp)
                        nc.tensor.matmul(psv[:], wv_sb[d][:], xt[:], start=st, stop=sp)

                    # RoPE on q heads and k (deinterleaved pair layout:
                    # swap partition halves, multiply by host-built tables)
                    H = P // 2
                    for h in range(NQH + 1):
                        src = psk if h == NQH else psq[h]
                        dst = kT if h == NQH else qT[h]
                        tsw = rp.tile([P, SC], F32, tag="tsw", name="tsw")
                        nc.scalar.activation(tsw[0:H, :], src[H:P, :], AF.Copy)
                        nc.scalar.activation(tsw[H:P, :], src[0:H, :], AF.Copy)
                        t1 = rp.tile([P, SC], F32, tag="t1", name="t1")
                        nc.vector.tensor_mul(t1[:], src[:], ropeC[:, ssl])
                        t2 = rp.tile([P, SC], F32, tag="t2", name="t2")
                        nc.vector.tensor_mul(t2[:], tsw[:], ropeS[:, ssl])
                        nc.vector.tensor_add(dst[:, ssl], t1[:], t2[:])

                    # V: transpose [hd, s] chunk into [s, hd] tiles
                    vtmp = rp.tile([P, SC], BF16, tag="vtmp", name="vtmp")
                    nc.scalar.activation(vtmp[:], psv[:], AF.Copy)
                    for t in range(SC // P):
                        ptr = ptrp.tile([P, P], BF16, tag="ptr", name="ptr")
                        nc.tensor.transpose(
                            ptr[:], vtmp[:, t * P:(t + 1) * P], ident[:])
                        nc.scalar.activation(
                            V[sc * (SC // P) + t][:], ptr[:], AF.Copy)

            # ---- phase 2: attention (transposed scores), AllGather per head ----
            dp = tc.alloc_tile_pool(name="dram", bufs=1, space="DRAM")
            with tc.tile_pool(name="mmul", bufs=1) as mmp, \
                 tc.tile_pool(name="E", bufs=2) as ep, \
                 tc.tile_pool(name="r2", bufs=2) as r2p, \
                 tc.tile_pool(name="pss", bufs=2, space="PSUM") as pssp, \
                 tc.tile_pool(name="psum", bufs=2, space="PSUM") as psump, \
                 tc.tile_pool(name="pav", bufs=2, space="PSUM") as pavp, \
                 tc.tile_pool(name="pb", bufs=1, space="PSUM") as pbp:
                cc_in = [dp.tile([P, SEQ], BF16, tag=f"cc_in{h}", name=f"cc_in{h}")
                         for h in range(NQH)]
                cc_out = [dp.tile([N_CORES * P, SEQ], BF16,
                                  tag=f"cc_out{h}", name=f"cc_out{h}")
                          for h in range(NQH)]
                mask_sb = [mmp.tile([P, SC], BF16, tag=f"mm{i}", name=f"mm{i}")
                           for i in range(max(n_mixed, 1))]
                for i in range(n_mixed):
                    nc.sync.dma_start(mask_sb[i][:], mm_d[i * P:(i + 1) * P, :])

                for h in range(NQH):
                    for qj in range(NSC):
                        qsl = slice(qj * SC, (qj + 1) * SC)
                        live = [ki for ki in range(NKT) if classes[ki, qj] != 2]
                        Es = []
                        for ki in live:
                            pss = pssp.tile([P, SC], F32, tag="pss", name="pss")
                            nc.tensor.matmul(
                                pss[:], kT[:, ki * P:(ki + 1) * P],
                                qT[h][:, qsl], start=True, stop=True)
                            e = ep.tile([P, SC], BF16, tag=f"E{ki}", name=f"E{ki}")
                            nc.scalar.activation(e[:], pss[:], AF.Exp, scale=scale)
                            if classes[ki, qj] == 1:
                                nc.vector.tensor_mul(
                                    e[:], e[:], mask_sb[mixed_idx[(ki, qj)]][:])
                            Es.append((ki, e))
                        psum = psump.tile([1, SC], F32, tag="psum", name="psum")
                        for i, (ki, e) in enumerate(Es):
                            nc.tensor.matmul(psum[:], ones_col[:], e[:],
                                             start=i == 0, stop=i == len(Es) - 1)
                        pav = pavp.tile([P, SC], F32, tag="pav", name="pav")
                        for i, (ki, e) in enumerate(Es):
                            nc.tensor.matmul(pav[:], V[ki][:], e[:],
                                             start=i == 0, stop=i == len(Es) - 1)
                        sums = r2p.tile([1, SC], F32, tag="sums", name="sums")
                        nc.scalar.activation(sums[:], psum[:], AF.Copy)
                        pb = pbp.tile([P, SC], F32, tag="pb", name="pb")
                        nc.tensor.matmul(pb[:], ones_row[:], sums[:],
                                         start=True, stop=True)
                        bsb = r2p.tile([P, SC], F32, tag="bsb", name="bsb")
                        nc.vector.reciprocal(bsb[:], pb[:])
                        nc.vector.tensor_mul(attnT[h][:, qsl], pav[:], bsb[:])
                    # ship this head while later heads compute (gpsimd/SWDGE:
                    # HWDGE DMAs concurrent with collectives hang in NRT)
                    nc.gpsimd.dma_start(cc_in[h][:], attnT[h][:])
                    nc.gpsimd.collective_compute(
                        "AllGather", mybir.AluOpType.bypass,
                        replica_groups=[list(range(N_CORES))],
                        ins=[cc_in[h].opt()], outs=[cc_out[h].opt()])

            # ---- phase 3: output projection columns ----
            # cc_out[h] row-tile j holds global head 4*j + h
            with tc.tile_pool(name="po", bufs=2, space="PSUM") as pop, \
                 tc.tile_pool(name="ob", bufs=2) as obp:
                NS4 = SC // P
                for w in range(4):
                    wsl = slice(w * SC, (w + 1) * SC)
                    ah = [ahp.tile([P, SC], BF16, tag=f"ah{c}", name=f"ah{c}")
                          for c in range(ND)]
                    for h in range(NQH):
                        for j in range(N_CORES):
                            nc.sync.dma_start(
                                ah[NQH * j + h][:],
                                cc_out[h][j * P:(j + 1) * P, wsl])
                    # h-outer accumulation: head 3 (latest AllGather) last, so
                    # its arrival hides under heads 0-2 matmuls; one weight
                    # load feeds 4 matmuls.
                    po = [pop.tile([P, QCOLS], F32, tag=f"po{s4}", name=f"po{s4}")
                          for s4 in range(NS4)]
                    for h in range(NQH):
                        for j in range(N_CORES):
                            c = NQH * j + h
                            for s4 in range(NS4):
                                nc.tensor.matmul(
                                    po[s4][:], ah[c][:, s4 * P:(s4 + 1) * P],
                                    wo_sb[c][:], start=h == 0 and j == 0,
                                    stop=h == NQH - 1 and j == N_CORES - 1)
                    for s4 in range(NS4):
                        st = w * NS4 + s4
                        ob = obp.tile([P, QCOLS], F32, tag="ob", name="ob")
                        nc.scalar.activation(ob[:], po[s4][:], AF.Copy)
                        nc.sync.dma_start(out_d[st * P:(st + 1) * P, :], ob[:])
            dp.release()

    _split_multi_waits(nc)
    return nc


def kernel(x, wq, wk, wv, wo, freqs_cos, freqs_sin, mask):
    x = np.asarray(x, dtype=np.float32)
    wq = np.asarray(wq, dtype=np.float32)
    wk = np.asarray(wk, dtype=np.float32)
    wv = np.asarray(wv, dtype=np.float32)
    wo = np.asarray(wo, dtype=np.float32)
    freqs_cos = np.asarray(freqs_cos, dtype=np.float32)
    freqs_sin = np.asarray(freqs_sin, dtype=np.float32)
    mask = np.asarray(mask, dtype=np.float32)

    bf = ml_dtypes.bfloat16
    # deinterleave head_dim pairs so RoPE becomes a partition-half swap
    perm = np.concatenate([np.arange(0, HD, 2), np.arange(1, HD, 2)])
    wq_p = wq.reshape(-1, HD, DIM)[:, perm, :].reshape(wq.shape)
    wk_p = wk.reshape(-1, HD, DIM)[:, perm, :].reshape(wk.shape)

    xT = np.ascontiguousarray(x[0].T).astype(bf)               # [DIM, SEQ]
    ropeC = np.ascontiguousarray(
        np.concatenate([freqs_cos.T, freqs_cos.T], axis=0))     # [128, SEQ]
    ropeS = np.ascontiguousarray(
        np.concatenate([-freqs_sin.T, freqs_sin.T], axis=0))

    classes, maskpack, mixed_idx = _classify_mask(mask)
    n_mixed = len(mixed_idx)

    nc = _build_program(classes, mixed_idx, n_mixed)

    in_maps = []
    for i in range(N_CORES):
        wqT = np.ascontiguousarray(
            wq_p[i * QCOLS:(i + 1) * QCOLS, :].T).astype(bf)    # [DIM, 512]
        wkT = np.ascontiguousarray(
            wk_p[i * HD:(i + 1) * HD, :].T).astype(bf)          # [DIM, 128]
        wvT = np.ascontiguousarray(
            wv[i * HD:(i + 1) * HD, :].T).astype(bf)
        # out[:, 512i:512(i+1)] = attn_full @ wo.T[:, 512i:...]
        woT = np.ascontiguousarray(
            wo[i * QCOLS:(i + 1) * QCOLS, :].T).astype(bf)      # [DIM, 512]
        in_maps.append({
            "xT": xT, "wqT": wqT, "wkT": wkT, "wvT": wvT, "woT": woT,
            "ropeC": ropeC, "ropeS": ropeS, "maskmul": maskpack,
        })

    res = run_bass_kernel_spmd(nc, in_maps, list(range(N_CORES)))
    global LAST_RESULT
    LAST_RESULT = res
    out = np.concatenate(
        [np.asarray(res.results[i]["out"]) for i in range(N_CORES)], axis=1)
    return out.reshape(1, SEQ, DIM).astype(np.float32)
